# revision 53
# baseline (speedup 1.0000x reference)
"""Trainium2 Bass kernel for nn_D2GroupConvolutionLayer (D2-equivariant GAT).

Math: per output view g and input view h the layer is a GAT with a GLOBAL
softmax over edges.  score(e) = u[src] + v[dst] factorizes, so the whole
gather -> softmax -> scatter collapses to dense algebra

    out_gh = diag(b) . M . diag(a) . H / (V * b^T M a)

with a = exp(u), b = exp(v) per-node scalars and M[d,s] the fixed
edge-multiplicity matrix (self-loops included, entries exact in fp8e4m3).

Sharding: data-parallel over the 8 (batch b, output view g) pairs.

Engine split (per core; h = 0..3 input views, 16 node-tiles each):
  PE   : H = x@W as 3 fp8 DoubleRow chains with residual compensation
         (x8a@W8a + x8a@W8r + x8r@W8a, lo*lo term dropped); uv score
         matmuls; G = M@(aH) as 8+8 fp8 DR main+residual chains per
         d-tile; tiny Ma matmuls for the softmax denominator z.
  ACT  : |H| (Abs), haugF16 = bf16(a*H) PSUM evacuation, exp batches,
         epilogue muls (odd d).
  DVE  : p2 = |H| (x) att (TT at 2x), u/v dots (tensor_scalar+accum at
         4x), haug8 = fp8 TensorCopy of haugF16 (2x), a8p copies,
         z-chain glue, view-3 epilogues.
  Pool : r8 = haugF16 - haug8 (TT, fp8 out), epilogue adds (odd d).

Pipeline: per view window, the G d-loop interleaves the next view's
A-phase tiles (2-per-step early: QSCHED), the lagged epilogues of the
current view, and the split Ma/z chain so no engine gates a boundary.
Out is written bf16 and widened to f32 on the host.
"""

import sys
from contextlib import ExitStack

for _p in ("/opt/trn_rl_repo/concourse", "/opt/trn_rl_repo"):
    if _p not in sys.path:
        sys.path.insert(0, _p)

import ml_dtypes  # noqa: E402
import numpy as np  # noqa: E402

import concourse.bass as bass  # noqa: E402
import concourse.bacc as bacc  # noqa: E402
import concourse.mybir as mybir  # noqa: E402
import concourse.tile as tile  # noqa: E402
import concourse.tile_utils as tile_utils  # noqa: E402
import bass_rust  # noqa: E402

B, V, N, F, O = 2, 4, 2048, 128, 512
NT = N // 128       # node tiles
NP = NT // 2        # DoubleRow s-pair steps
F32, BF16 = mybir.dt.float32, mybir.dt.bfloat16
FP8 = mybir.dt.float8e4
E4M3 = ml_dtypes.float8_e4m3
DR = mybir.MatmulPerfMode.DoubleRow
MUL, ADD, SUB, MAX = (mybir.AluOpType.mult, mybir.AluOpType.add,
                      mybir.AluOpType.subtract, mybir.AluOpType.max)

tile_utils.max_sbuf_usage = 204 * 1024


class _TileContext(tile.TileContext):
    """Split the exit-drain's sem waits across single-wait carrier nops
    (walrus caps sync waits at 1/instruction)."""

    def _drain_and_barrier(self, tick_clock, wait_clock):
        nc = self.nc
        probe = nc.sync.nop(nofuse=True)
        wait_clock.add_sem_waits(
            probe.ins, bass_rust.ScopedClock({None: tick_clock.global_clock})
        )
        si = probe.ins.sync_info
        if si is not None and si.on_wait and len(si.on_wait) > 1:
            waits = list(si.on_wait)
            si.on_wait = [waits[0]]
            for w in waits[1:]:
                carrier = nc.sync.nop(nofuse=True)
                carrier.ins.sync_info = mybir.SyncInfo(on_wait=[w], on_update=[])
        nc.sync.drain()
        nc.all_engine_barrier()
        popped = nc._tile_sem_poison_stack.pop()
        assert popped is self._sem_poison
        nc.clear_and_free_semaphores(list(self.sems.allocated().values()))
        nc.all_engine_barrier()


def _build_program():
    nc = bacc.Bacc("TRN2", target_bir_lowering=False, debug=False)

    x8a_d = nc.dram_tensor("x8a", [V, 2, 128, N], FP8, kind="ExternalInput").ap()
    x8r_d = nc.dram_tensor("x8r", [V, 2, 128, N], FP8, kind="ExternalInput").ap()
    w8a_d = nc.dram_tensor("w8a", [V, 2, 128, O], FP8, kind="ExternalInput").ap()
    w8r_d = nc.dram_tensor("w8r", [V, 2, 128, O], FP8, kind="ExternalInput").ap()
    wuv_d = nc.dram_tensor("wuv", [V, 2, 128, 2], BF16, kind="ExternalInput").ap()
    mt8_d = nc.dram_tensor("mt8", [128, NP, 2, N], FP8, kind="ExternalInput").ap()
    attb_d = nc.dram_tensor("attb", [128, 2 * O], BF16, kind="ExternalInput").ap()
    biasb_d = nc.dram_tensor("biasb", [128, O], BF16, kind="ExternalInput").ap()
    out_d = nc.dram_tensor("out", [NT, 128, O], BF16, kind="ExternalOutput").ap()

    with ExitStack() as ctx:
        tc = ctx.enter_context(_TileContext(nc))
        pool = ctx.enter_context(tc.tile_pool(name="main", bufs=1))
        h8pool = ctx.enter_context(tc.tile_pool(name="h8", bufs=2))
        stpool = ctx.enter_context(tc.tile_pool(name="st", bufs=2))
        tmpool = ctx.enter_context(tc.tile_pool(name="tm", bufs=8))
        pp = ctx.enter_context(tc.tile_pool(name="ps", bufs=1, space="PSUM"))
        pph = ctx.enter_context(tc.tile_pool(name="psh", bufs=4, space="PSUM"))
        ppg = ctx.enter_context(tc.tile_pool(name="psg", bufs=3, space="PSUM"))

        # ---- persistent SBUF ----
        x8a = pool.tile([128, V, 2, N], FP8)
        x8r = pool.tile([128, V, 2, N], FP8)
        w8a = pool.tile([128, V, 2, O], FP8)
        w8r = pool.tile([128, V, 2, O], FP8)
        wuv = pool.tile([128, V, 2, 2], BF16)
        mt8 = pool.tile([128, NP, 2, N], FP8)
        attb = pool.tile([128, 2 * O], BF16)
        biasb = pool.tile([128, O], BF16)
        out_acc = pool.tile([128, NT, O], BF16)
        ones4 = pool.tile([128, 1], F32)   # value V=4 -> pz = V*z
        ones_row = pool.tile([1, 128], F32)
        z1 = pool.tile([1, V], F32)
        # one PSUM bank: per-parity 96 cols = [0:64] z-scratch, [64:96] uv
        zbig = pp.tile([128, 2, 96], F32)

        # ---- DMA: view-0 critical path first, issued from parallel queues ----
        NQ = N // 4
        nc.sync.dma_start(x8a[:, 0, :, bass.ts(0, NQ)],
                          x8a_d[0, :, :, bass.ts(0, NQ)])
        nc.scalar.dma_start(w8a[:, 0], w8a_d[0])
        nc.gpsimd.dma_start(w8r[:, 0], w8r_d[0])
        nc.sync.dma_start(x8r[:, 0, :, bass.ts(0, NQ)],
                          x8r_d[0, :, :, bass.ts(0, NQ)])
        nc.gpsimd.dma_start(wuv[:, 0], wuv_d[0])
        nc.gpsimd.dma_start(attb[:], attb_d[:])
        for q in range(1, 4):
            nc.sync.dma_start(x8a[:, 0, :, bass.ts(q, NQ)],
                              x8a_d[0, :, :, bass.ts(q, NQ)])
            nc.sync.dma_start(x8r[:, 0, :, bass.ts(q, NQ)],
                              x8r_d[0, :, :, bass.ts(q, NQ)])
        for h in range(1, V):
            nc.sync.dma_start(x8a[:, h], x8a_d[h])
            nc.sync.dma_start(x8r[:, h], x8r_d[h])
            nc.sync.dma_start(w8a[:, h], w8a_d[h])
            nc.sync.dma_start(w8r[:, h], w8r_d[h])
            nc.sync.dma_start(wuv[:, h], wuv_d[h])
        for j in range(NP):
            nc.sync.dma_start(mt8[:, j, :, :], mt8_d[:, j])
        nc.sync.dma_start(biasb[:], biasb_d[:])

        nc.vector.memset(ones4[:], float(V))
        nc.vector.memset(ones_row[:], 1.0)

        st = {}
        phs = {}

        def a_open(h):
            haug8 = h8pool.tile([128, NP, 2, O], FP8, tag="h8", name=f"h8_{h}")
            r8 = h8pool.tile([128, NP, 2, O], FP8, tag="r8", name=f"r8_{h}")
            udot = stpool.tile([128, NT], F32, tag="ud", name=f"ud{h}")
            vdot = stpool.tile([128, NT], F32, tag="vd", name=f"vd{h}")
            uvsb = stpool.tile([128, NT, 2], F32, tag="uv", name=f"uv{h}")
            a_st = stpool.tile([128, NT], F32, tag="as", name=f"as{h}")
            cu = stpool.tile([128, NT], F32, tag="cu", name=f"cu{h}")
            a8p = stpool.tile([128, NP, 2, 1], FP8, tag="a8", name=f"a8_{h}")
            st[h] = dict(haug8=haug8, r8=r8, udot=udot, vdot=vdot,
                         uvsb=uvsb, a_st=a_st, cu=cu, a8p=a8p, sh=h % 2)

        def a_tile(h, t):
            """One A-phase tile: PE H-matmuls (3 fp8 DR chains + 4 uv),
            ACT |H|, DVE att-dots; per odd t the lagged pair quantization:
            exp pair -> haugF16 (ACT) -> haug8 (DVE copy) -> r8 (Pool)."""
            s = st[h]
            blk = bass.ts(t, 128)
            ph = pph.tile([128, O], F32, tag="hps", name=f"ph{h}_{t}")
            phs[(h, t)] = ph
            nc.tensor.matmul(ph[:], x8a[:, h, :, blk], w8a[:, h], start=True,
                             stop=False, perf_mode=DR)
            nc.tensor.matmul(ph[:], x8a[:, h, :, blk], w8r[:, h], start=False,
                             stop=False, perf_mode=DR)
            nc.tensor.matmul(ph[:], x8r[:, h, :, blk], w8a[:, h], start=False,
                             stop=True, perf_mode=DR)
            puv = zbig[:, s["sh"], 64 + 2 * t:66 + 2 * t]
            nc.tensor.matmul(puv, x8a[:, h, 0, blk], wuv[:, h, 0],
                             start=True, stop=False)
            nc.tensor.matmul(puv, x8a[:, h, 1, blk], wuv[:, h, 1],
                             start=False, stop=False)
            nc.tensor.matmul(puv, x8r[:, h, 0, blk], wuv[:, h, 0],
                             start=False, stop=False)
            nc.tensor.matmul(puv, x8r[:, h, 1, blk], wuv[:, h, 1],
                             start=False, stop=True)
            habs = tmpool.tile([128, 1, O], BF16, tag="habs", name=f"hab{h}_{t}")
            nc.scalar.activation(habs[:, 0, :], ph[:],
                                 mybir.ActivationFunctionType.Abs)
            p2 = tmpool.tile([128, 2 * O], BF16, tag="p2", name=f"p2_{h}_{t}")
            nc.vector.tensor_tensor(p2[:], habs[:].broadcast_to((128, 2, O)),
                                    attb[:], op=MUL)
            scr = tmpool.tile([128, O], BF16, tag="scr", name=f"sc{h}_{t}")
            nc.vector.tensor_scalar(scr[:], p2[:, :O], 1.0, 0.0,
                                    op0=MUL, op1=ADD,
                                    accum_out=s["udot"][:, t:t + 1])
            scr2 = tmpool.tile([128, O], BF16, tag="scr2", name=f"sd{h}_{t}")
            nc.vector.tensor_scalar(scr2[:], p2[:, O:], 1.0, 0.0,
                                    op0=MUL, op1=ADD,
                                    accum_out=s["vdot"][:, t:t + 1])
            if True:
                nc.scalar.copy(s["uvsb"][:, t:t + 1, :],
                               zbig[:, s["sh"], 64 + 2 * t:66 + 2 * t])
                nc.vector.scalar_tensor_tensor(
                    s["cu"][:, t:t + 1], s["udot"][:, t:t + 1], 0.4,
                    s["uvsb"][:, t:t + 1, 0], op0=MUL, op1=ADD)
                nc.scalar.activation(s["a_st"][:, t:t + 1],
                                     s["cu"][:, t:t + 1],
                                     mybir.ActivationFunctionType.Exp)
                for tt in (t,):
                    j, i2 = tt // 2, tt % 2
                    phv = phs.pop((h, tt))
                    hf = tmpool.tile([128, O], BF16, tag="hf",
                                     name=f"hf{h}_{tt}")
                    nc.scalar.mul(hf[:], phv[:], s["a_st"][:, tt:tt + 1])
                    nc.vector.tensor_copy(s["haug8"][:, j, i2, :], hf[:])
                    nc.gpsimd.tensor_tensor(s["r8"][:, j, i2, :], hf[:],
                                            s["haug8"][:, j, i2, :], op=SUB)

        def a_close(h):
            """b = exp(0.4*vdot + vlin), a8p fp8 copy, Ma matmuls for z."""
            s = st[h]
            b_st = stpool.tile([128, NT], F32, tag="bs", name=f"bs{h}")
            cv = stpool.tile([128, NT], F32, tag="cv", name=f"cv{h}")
            nc.vector.scalar_tensor_tensor(cv[:], s["vdot"][:], 0.4,
                                           s["uvsb"][:, :, 1], op0=MUL, op1=ADD)
            nc.scalar.activation(b_st[:], cv[:],
                                 mybir.ActivationFunctionType.Exp)
            ma_half(h, 1)
            s["b_st"] = b_st

        def a8p_copy(h, half):
            s = st[h]
            j0 = half * (NP // 2)
            nc.vector.tensor_copy(s["a8p"][:, j0:j0 + NP // 2, :, 0],
                                  s["a_st"][:, j0 * 2:(j0 + NP // 2) * 2])

        def ma_half(h, half):
            """Half of the z Ma-chain: 4 j-steps of Ma accumulation."""
            s = st[h]
            sh = s["sh"]
            j0 = half * (NP // 2)
            for j in range(j0, j0 + NP // 2):
                for d in range(NT):
                    nc.tensor.matmul(
                        zbig[:, sh, d:d + 1], mt8[:, j, :, bass.ts(d, 128)],
                        s["a8p"][:, j, :, :],
                        start=(j == 0 and d == 0),
                        stop=(j == NP - 1 and d == NT - 1),
                        perf_mode=DR, skip_group_check=True)

        def z_chain(h):
            """z = b^T(M a8), rz = 1/(V z), bp = b*rz."""
            s = st[h]
            sh, b_st = s["sh"], s["b_st"]
            zcol = stpool.tile([128, 1], F32, tag="zc", name=f"zc{h}")
            zscr = stpool.tile([128, NT], F32, tag="zs", name=f"zs{h}")
            nc.vector.scalar_tensor_tensor(
                zscr[:], zbig[:, sh, 0:NT], 1.0, b_st[:],
                op0=MUL, op1=MUL, accum_out=zcol[:])
            nc.tensor.matmul(zbig[0:1, sh, 32:33], ones4[:], zcol[:],
                             start=False, stop=True, skip_group_check=True)
            nc.vector.reciprocal(z1[0:1, h:h + 1], zbig[0:1, sh, 32:33])
            nc.tensor.matmul(zbig[:, sh, 33:34], ones_row[:], z1[0:1, h:h + 1],
                             start=False, stop=True, skip_group_check=True)
            bp = stpool.tile([128, NT], F32, tag="bp", name=f"bp{h}")
            nc.vector.tensor_scalar(bp[:], b_st[:], zbig[:, sh, 33:34], None,
                                    op0=MUL)
            s["bp"] = bp

        pgs = {}

        def g_epi(h, d, split=None):
            """Lagged epilogue for d-tile: out_acc[d] (+)= bp[d] * pg."""
            bp = st[h]["bp"]
            pg = (pgs.pop((h, d)) if split in (None, 1)
                  else pgs[(h, d)])
            last = h == V - 1
            prev = biasb[:] if h == 0 else out_acc[:, d, :]
            cols = slice(None)
            if split is not None:
                cols = slice(0, 448) if split == 0 else slice(448, O)
            if last:
                outf = tmpool.tile([128, O], BF16, tag="outf",
                                   name=f"of{d}_{split}")
            if d % 2 == 1 and not last:
                # ACT-mul + Pool-add path
                tmpd = tmpool.tile([128, O], BF16, tag="tmpd", name=f"tm{h}_{d}")
                nc.scalar.mul(tmpd[:], pg[:], bp[:, d:d + 1])
                nc.gpsimd.tensor_tensor(out_acc[:, d, :], tmpd[:],
                                        prev[:], op=ADD)
            else:
                nc.vector.scalar_tensor_tensor(
                    outf[:, cols] if last else out_acc[:, d, cols],
                    pg[:, cols], bp[:, d:d + 1],
                    prev[:, cols],
                    op0=MUL, op1=ADD)
            if last:
                nc.sync.dma_start(out_d[d, :, cols], outf[:, cols])

        # quantization tile schedule per G d-step: 2 tiles/step early so the
        # next view's quant finishes well before the window boundary
        QSCHED = {0: (0, 1), 1: (2,), 2: (3, 4), 3: (5,),
                  4: (6, 7), 5: (8,), 6: (9, 10), 7: (11,), 8: (12,),
                  9: (13,), 10: (14,), 11: (15,)}

        def g_tile(h, d, hn):
            """B-phase d-tile: 8 main + 8 residual DR matmuls into one bank,
            then the next view's A-phase tiles, then the lagged epilogue.
            The very last tile runs as two column-group chains so its
            epilogue + out-DMA overlap the second chain."""
            s = st[h]
            pg = ppg.tile([128, O], F32, tag="pg", name=f"pg{h}_{d}")
            pgs[(h, d)] = pg
            blk = bass.ts(d, 128)
            for j in range(NP):
                nc.tensor.matmul(pg[:], mt8[:, j, :, blk],
                                 s["haug8"][:, j, :, :],
                                 start=(j == 0), stop=False, perf_mode=DR)
                nc.tensor.matmul(pg[:], mt8[:, j, :, blk],
                                 s["r8"][:, j, :, :],
                                 start=False, stop=(j == NP - 1),
                                 perf_mode=DR)
            if hn is not None:
                for t in QSCHED.get(d, ()):
                    a_tile(hn, t)
                if d == 11:
                    a8p_copy(hn, 0)
                    ma_half(hn, 0)
                if d == 12:
                    a8p_copy(hn, 1)
            if d > 0:
                g_epi(h, d - 1)
            if d == NT - 1:
                if h == V - 1:
                    g_epi(h, d, split=0)
                    g_epi(h, d, split=1)
                else:
                    g_epi(h, d)
                if hn is not None:
                    a_close(hn)

        # ================= trace =================
        a_open(0)
        for t in range(NT):
            a_tile(0, t)
        a8p_copy(0, 0)
        ma_half(0, 0)
        a8p_copy(0, 1)
        a_close(0)
        for h in range(V):
            if h + 1 < V:
                a_open(h + 1)
            for d in range(NT):
                g_tile(h, d, h + 1 if h + 1 < V else None)
                if d == 0:
                    z_chain(h)

    nc.compile()
    _dedup_ldweights(nc)
    return nc


def _dedup_ldweights(nc):
    """Drop InstLdweights that reload the weights AP already resident (the
    main+residual matmul pairs share one mt8 block)."""
    pe = mybir.EngineType.PE
    removed = 0
    for bb in nc.m.functions[0].blocks:
        insts = list(bb.instructions)
        out = []
        last_key = None
        for i in insts:
            ty = type(i).__name__
            if ty == "InstLdweights":
                ap = i.ins[0]
                key = (str(ap.memref), ap.offset, str(ap.ap))
                si = i.sync_info
                clean = si is None or (not si.on_wait and not si.on_update)
                if key == last_key and clean:
                    removed += 1
                    continue
                last_key = key
            elif getattr(i, "engine", None) == pe:
                if ty == "InstMatmult":
                    try:
                        ap = i.ins[1]
                        mk = (str(ap.memref), ap.offset, str(ap.ap))
                    except Exception:
                        mk = None
                    if mk != last_key:
                        last_key = None
                else:
                    last_key = None
            out.append(i)
        if removed:
            bb.instructions = out
    return removed


_SIGNS = None


def _signs():
    global _SIGNS
    if _SIGNS is None:
        s = np.ones((4, F), dtype=np.float32)
        for r in range(4):
            if r & 1:
                s[r, [0, 2]] = -1.0
            if r & 2:
                s[r, [1, 3]] = -1.0
        _SIGNS = s
    return _SIGNS


def _q8pair(x):
    """fp8 hi/lo split of an array (returns e4m3 hi, e4m3 residual)."""
    hi = x.astype(E4M3)
    lo = (x - hi.astype(np.float32)).astype(E4M3)
    return hi, lo


def _host_prep(x, edge_index, W, att, bias):
    """Pure relayout/quantization preprocessing."""
    signs = _signs()
    x = np.ascontiguousarray(x, dtype=np.float32)
    W = np.asarray(W, dtype=np.float32)
    att = np.asarray(att, dtype=np.float32).reshape(2 * O)
    bias = np.asarray(bias, dtype=np.float32)
    ei = np.asarray(edge_index)

    M = np.zeros((N, N), dtype=np.float32)
    np.add.at(M, (ei[1], ei[0]), 1.0)
    M[np.arange(N), np.arange(N)] += 1.0
    # mt8[p, j, i, d] = M[d, (2j+i)*128 + p]
    MT = np.ascontiguousarray(M.T).reshape(NP, 2, 128, N)
    mt8 = np.ascontiguousarray(MT.transpose(2, 0, 1, 3).astype(E4M3))

    att_u, att_v = att[:O], att[O:]
    W1, W2 = W[:F], W[F:]
    attb = np.ascontiguousarray(
        np.broadcast_to(att, (128, 2 * O))).astype(ml_dtypes.bfloat16)
    biasb = np.ascontiguousarray(
        np.broadcast_to(bias, (128, O))).astype(ml_dtypes.bfloat16)

    # bf16 x, transposed to [B, V, F, N]
    xT = np.ascontiguousarray(x.transpose(0, 1, 3, 2)).astype(
        ml_dtypes.bfloat16).astype(np.float32)

    in_maps = []
    for core in range(8):
        b, g = divmod(core, V)
        x8a = np.empty((V, 2, 128, N), dtype=E4M3)
        x8r = np.empty((V, 2, 128, N), dtype=E4M3)
        w8a = np.empty((V, 2, 128, O), dtype=E4M3)
        w8r = np.empty((V, 2, 128, O), dtype=E4M3)
        wuvc = np.empty((V, 2, 128, 2), dtype=ml_dtypes.bfloat16)
        for h in range(V):
            w1s = signs[h ^ g][:, None] * W1
            w2s = signs[h][:, None] * W2
            w1s16 = w1s.astype(ml_dtypes.bfloat16).astype(np.float32)
            w2s16 = w2s.astype(ml_dtypes.bfloat16).astype(np.float32)
            x8a[h, 0], x8r[h, 0] = _q8pair(xT[b, h])
            x8a[h, 1], x8r[h, 1] = _q8pair(xT[b, g ^ h])
            w8a[h, 0], w8r[h, 0] = _q8pair(w1s16)
            w8a[h, 1], w8r[h, 1] = _q8pair(w2s16)
            wuvc[h, 0, :, 0] = 0.6 * (w1s @ att_u)
            wuvc[h, 0, :, 1] = 0.6 * (w1s @ att_v)
            wuvc[h, 1, :, 0] = 0.6 * (w2s @ att_u)
            wuvc[h, 1, :, 1] = 0.6 * (w2s @ att_v)
        in_maps.append({
            "x8a": x8a, "x8r": x8r, "w8a": w8a, "w8r": w8r, "wuv": wuvc,
            "mt8": mt8, "attb": attb, "biasb": biasb,
        })
    return in_maps


_NC = None


def kernel(x, edge_index, W, att, bias):
    global _NC
    if _NC is None:
        _NC = _build_program()
    in_maps = _host_prep(x, edge_index, W, att, bias)

    from concourse.bass_utils import run_bass_kernel_spmd

    res = run_bass_kernel_spmd(_NC, in_maps, list(range(8)))
    out = np.empty((B, V, N, O), dtype=np.float32)
    for core in range(8):
        b, g = divmod(core, V)
        out[b, g] = np.asarray(res.results[core]["out"], dtype=np.float32).reshape(N, O)
    return out


# revision 54
# speedup vs baseline: 1.0004x; 1.0004x over previous
"""Trainium2 Bass kernel for nn_D2GroupConvolutionLayer (D2-equivariant GAT).

Math: per output view g and input view h the layer is a GAT with a GLOBAL
softmax over edges.  score(e) = u[src] + v[dst] factorizes, so the whole
gather -> softmax -> scatter collapses to dense algebra

    out_gh = diag(b) . M . diag(a) . H / (V * b^T M a)

with a = exp(u), b = exp(v) per-node scalars and M[d,s] the fixed
edge-multiplicity matrix (self-loops included, entries exact in fp8e4m3).

Sharding: data-parallel over the 8 (batch b, output view g) pairs.

Engine split (per core; h = 0..3 input views, 16 node-tiles each):
  PE   : H = x@W as 3 fp8 DoubleRow chains with residual compensation
         (x8a@W8a + x8a@W8r + x8r@W8a, lo*lo term dropped); uv score
         matmuls; G = M@(aH) as 8+8 fp8 DR main+residual chains per
         d-tile; tiny Ma matmuls for the softmax denominator z.
  ACT  : |H| (Abs), haugF16 = bf16(a*H) PSUM evacuation, exp batches,
         epilogue muls (odd d).
  DVE  : p2 = |H| (x) att (TT at 2x), u/v dots (tensor_scalar+accum at
         4x), haug8 = fp8 TensorCopy of haugF16 (2x), a8p copies,
         z-chain glue, view-3 epilogues.
  Pool : r8 = haugF16 - haug8 (TT, fp8 out), epilogue adds (odd d).

Pipeline: per view window, the G d-loop interleaves the next view's
A-phase tiles (2-per-step early: QSCHED), the lagged epilogues of the
current view, and the split Ma/z chain so no engine gates a boundary.
Out is written bf16 and widened to f32 on the host.
"""

import sys
from contextlib import ExitStack

for _p in ("/opt/trn_rl_repo/concourse", "/opt/trn_rl_repo"):
    if _p not in sys.path:
        sys.path.insert(0, _p)

import ml_dtypes  # noqa: E402
import numpy as np  # noqa: E402

import concourse.bass as bass  # noqa: E402
import concourse.bacc as bacc  # noqa: E402
import concourse.mybir as mybir  # noqa: E402
import concourse.tile as tile  # noqa: E402
import concourse.tile_utils as tile_utils  # noqa: E402
import bass_rust  # noqa: E402

B, V, N, F, O = 2, 4, 2048, 128, 512
NT = N // 128       # node tiles
NP = NT // 2        # DoubleRow s-pair steps
F32, BF16 = mybir.dt.float32, mybir.dt.bfloat16
FP8 = mybir.dt.float8e4
E4M3 = ml_dtypes.float8_e4m3
DR = mybir.MatmulPerfMode.DoubleRow
MUL, ADD, SUB, MAX = (mybir.AluOpType.mult, mybir.AluOpType.add,
                      mybir.AluOpType.subtract, mybir.AluOpType.max)

tile_utils.max_sbuf_usage = 204 * 1024


class _TileContext(tile.TileContext):
    """Split the exit-drain's sem waits across single-wait carrier nops
    (walrus caps sync waits at 1/instruction)."""

    def _drain_and_barrier(self, tick_clock, wait_clock):
        nc = self.nc
        probe = nc.sync.nop(nofuse=True)
        wait_clock.add_sem_waits(
            probe.ins, bass_rust.ScopedClock({None: tick_clock.global_clock})
        )
        si = probe.ins.sync_info
        if si is not None and si.on_wait and len(si.on_wait) > 1:
            waits = list(si.on_wait)
            si.on_wait = [waits[0]]
            for w in waits[1:]:
                carrier = nc.sync.nop(nofuse=True)
                carrier.ins.sync_info = mybir.SyncInfo(on_wait=[w], on_update=[])
        nc.sync.drain()
        nc.all_engine_barrier()
        popped = nc._tile_sem_poison_stack.pop()
        assert popped is self._sem_poison
        nc.clear_and_free_semaphores(list(self.sems.allocated().values()))
        nc.all_engine_barrier()


def _build_program():
    nc = bacc.Bacc("TRN2", target_bir_lowering=False, debug=False)

    x8a_d = nc.dram_tensor("x8a", [V, 2, 128, N], FP8, kind="ExternalInput").ap()
    x8r_d = nc.dram_tensor("x8r", [V, 2, 128, N], FP8, kind="ExternalInput").ap()
    w8a_d = nc.dram_tensor("w8a", [V, 2, 128, O], FP8, kind="ExternalInput").ap()
    w8r_d = nc.dram_tensor("w8r", [V, 2, 128, O], FP8, kind="ExternalInput").ap()
    wuv_d = nc.dram_tensor("wuv", [V, 2, 128, 2], BF16, kind="ExternalInput").ap()
    mt8_d = nc.dram_tensor("mt8", [128, NP, 2, N], FP8, kind="ExternalInput").ap()
    attb_d = nc.dram_tensor("attb", [128, 2 * O], BF16, kind="ExternalInput").ap()
    biasb_d = nc.dram_tensor("biasb", [128, O], BF16, kind="ExternalInput").ap()
    out_d = nc.dram_tensor("out", [NT, 128, O], BF16, kind="ExternalOutput").ap()

    with ExitStack() as ctx:
        tc = ctx.enter_context(_TileContext(nc))
        pool = ctx.enter_context(tc.tile_pool(name="main", bufs=1))
        h8pool = ctx.enter_context(tc.tile_pool(name="h8", bufs=2))
        stpool = ctx.enter_context(tc.tile_pool(name="st", bufs=2))
        tmpool = ctx.enter_context(tc.tile_pool(name="tm", bufs=8))
        pp = ctx.enter_context(tc.tile_pool(name="ps", bufs=1, space="PSUM"))
        pph = ctx.enter_context(tc.tile_pool(name="psh", bufs=4, space="PSUM"))
        ppg = ctx.enter_context(tc.tile_pool(name="psg", bufs=3, space="PSUM"))

        # ---- persistent SBUF ----
        x8a = pool.tile([128, V, 2, N], FP8)
        x8r = pool.tile([128, V, 2, N], FP8)
        w8a = pool.tile([128, V, 2, O], FP8)
        w8r = pool.tile([128, V, 2, O], FP8)
        wuv = pool.tile([128, V, 2, 2], BF16)
        mt8 = pool.tile([128, NP, 2, N], FP8)
        attb = pool.tile([128, 2 * O], BF16)
        biasb = pool.tile([128, O], BF16)
        out_acc = pool.tile([128, NT, O], BF16)
        ones4 = pool.tile([128, 1], F32)   # value V=4 -> pz = V*z
        ones_row = pool.tile([1, 128], F32)
        z1 = pool.tile([1, V], F32)
        # one PSUM bank: per-parity 96 cols = [0:64] z-scratch, [64:96] uv
        zbig = pp.tile([128, 2, 96], F32)

        # ---- DMA: view-0 critical path first, issued from parallel queues ----
        NQ = N // 4
        nc.sync.dma_start(x8a[:, 0, :, bass.ts(0, NQ)],
                          x8a_d[0, :, :, bass.ts(0, NQ)])
        nc.scalar.dma_start(w8a[:, 0], w8a_d[0])
        nc.gpsimd.dma_start(w8r[:, 0], w8r_d[0])
        nc.sync.dma_start(x8r[:, 0, :, bass.ts(0, NQ)],
                          x8r_d[0, :, :, bass.ts(0, NQ)])
        nc.gpsimd.dma_start(wuv[:, 0], wuv_d[0])
        nc.gpsimd.dma_start(attb[:], attb_d[:])
        for q in range(1, 4):
            nc.sync.dma_start(x8a[:, 0, :, bass.ts(q, NQ)],
                              x8a_d[0, :, :, bass.ts(q, NQ)])
            nc.sync.dma_start(x8r[:, 0, :, bass.ts(q, NQ)],
                              x8r_d[0, :, :, bass.ts(q, NQ)])
        for h in range(1, V):
            nc.sync.dma_start(x8a[:, h], x8a_d[h])
            nc.sync.dma_start(x8r[:, h], x8r_d[h])
            nc.sync.dma_start(w8a[:, h], w8a_d[h])
            nc.sync.dma_start(w8r[:, h], w8r_d[h])
            nc.sync.dma_start(wuv[:, h], wuv_d[h])
        for j in range(NP):
            nc.sync.dma_start(mt8[:, j, :, :], mt8_d[:, j])
        nc.sync.dma_start(biasb[:], biasb_d[:])

        nc.vector.memset(ones4[:], float(V))
        nc.vector.memset(ones_row[:], 1.0)

        st = {}
        phs = {}

        def a_open(h):
            haug8 = h8pool.tile([128, NP, 2, O], FP8, tag="h8", name=f"h8_{h}")
            r8 = h8pool.tile([128, NP, 2, O], FP8, tag="r8", name=f"r8_{h}")
            udot = stpool.tile([128, NT], F32, tag="ud", name=f"ud{h}")
            vdot = stpool.tile([128, NT], F32, tag="vd", name=f"vd{h}")
            uvsb = stpool.tile([128, NT, 2], F32, tag="uv", name=f"uv{h}")
            a_st = stpool.tile([128, NT], F32, tag="as", name=f"as{h}")
            cu = stpool.tile([128, NT], F32, tag="cu", name=f"cu{h}")
            a8p = stpool.tile([128, NP, 2, 1], FP8, tag="a8", name=f"a8_{h}")
            st[h] = dict(haug8=haug8, r8=r8, udot=udot, vdot=vdot,
                         uvsb=uvsb, a_st=a_st, cu=cu, a8p=a8p, sh=h % 2)

        def a_tile(h, t):
            """One A-phase tile: PE H-matmuls (3 fp8 DR chains + 4 uv),
            ACT |H|, DVE att-dots; per odd t the lagged pair quantization:
            exp pair -> haugF16 (ACT) -> haug8 (DVE copy) -> r8 (Pool)."""
            s = st[h]
            blk = bass.ts(t, 128)
            ph = pph.tile([128, O], F32, tag="hps", name=f"ph{h}_{t}")
            phs[(h, t)] = ph
            nc.tensor.matmul(ph[:], x8a[:, h, :, blk], w8a[:, h], start=True,
                             stop=False, perf_mode=DR)
            nc.tensor.matmul(ph[:], x8a[:, h, :, blk], w8r[:, h], start=False,
                             stop=False, perf_mode=DR)
            nc.tensor.matmul(ph[:], x8r[:, h, :, blk], w8a[:, h], start=False,
                             stop=True, perf_mode=DR)
            puv = zbig[:, s["sh"], 64 + 2 * t:66 + 2 * t]
            nc.tensor.matmul(puv, x8a[:, h, 0, blk], wuv[:, h, 0],
                             start=True, stop=False)
            nc.tensor.matmul(puv, x8a[:, h, 1, blk], wuv[:, h, 1],
                             start=False, stop=False)
            nc.tensor.matmul(puv, x8r[:, h, 0, blk], wuv[:, h, 0],
                             start=False, stop=False)
            nc.tensor.matmul(puv, x8r[:, h, 1, blk], wuv[:, h, 1],
                             start=False, stop=True)
            habs = tmpool.tile([128, 1, O], BF16, tag="habs", name=f"hab{h}_{t}")
            nc.scalar.activation(habs[:, 0, :], ph[:],
                                 mybir.ActivationFunctionType.Abs)
            p2 = tmpool.tile([128, 2 * O], BF16, tag="p2", name=f"p2_{h}_{t}")
            nc.vector.tensor_tensor(p2[:], habs[:].broadcast_to((128, 2, O)),
                                    attb[:], op=MUL)
            scr = tmpool.tile([128, O], BF16, tag="scr", name=f"sc{h}_{t}")
            nc.vector.tensor_scalar(scr[:], p2[:, :O], 1.0, 0.0,
                                    op0=MUL, op1=ADD,
                                    accum_out=s["udot"][:, t:t + 1])
            scr2 = tmpool.tile([128, O], BF16, tag="scr2", name=f"sd{h}_{t}")
            nc.vector.tensor_scalar(scr2[:], p2[:, O:], 1.0, 0.0,
                                    op0=MUL, op1=ADD,
                                    accum_out=s["vdot"][:, t:t + 1])
            if t % 2 == 1:
                nc.scalar.copy(s["uvsb"][:, t - 1:t + 1, :],
                               zbig[:, s["sh"], 62 + 2 * t:66 + 2 * t])
                nc.vector.scalar_tensor_tensor(
                    s["cu"][:, t - 1:t + 1], s["udot"][:, t - 1:t + 1], 0.4,
                    s["uvsb"][:, t - 1:t + 1, 0], op0=MUL, op1=ADD)
                nc.scalar.activation(s["a_st"][:, t - 1:t + 1],
                                     s["cu"][:, t - 1:t + 1],
                                     mybir.ActivationFunctionType.Exp)
                for tt in (t - 1, t):
                    j, i2 = tt // 2, tt % 2
                    phv = phs.pop((h, tt))
                    hf = tmpool.tile([128, O], BF16, tag="hf",
                                     name=f"hf{h}_{tt}")
                    nc.scalar.mul(hf[:], phv[:], s["a_st"][:, tt:tt + 1])
                    nc.vector.tensor_copy(s["haug8"][:, j, i2, :], hf[:])
                    nc.gpsimd.tensor_tensor(s["r8"][:, j, i2, :], hf[:],
                                            s["haug8"][:, j, i2, :], op=SUB)

        def a_close(h):
            """b = exp(0.4*vdot + vlin), a8p fp8 copy, Ma matmuls for z."""
            s = st[h]
            b_st = stpool.tile([128, NT], F32, tag="bs", name=f"bs{h}")
            cv = stpool.tile([128, NT], F32, tag="cv", name=f"cv{h}")
            nc.vector.scalar_tensor_tensor(cv[:], s["vdot"][:], 0.4,
                                           s["uvsb"][:, :, 1], op0=MUL, op1=ADD)
            nc.scalar.activation(b_st[:], cv[:],
                                 mybir.ActivationFunctionType.Exp)
            ma_half(h, 1)
            s["b_st"] = b_st

        def a8p_copy(h, half):
            s = st[h]
            j0 = half * (NP // 2)
            nc.vector.tensor_copy(s["a8p"][:, j0:j0 + NP // 2, :, 0],
                                  s["a_st"][:, j0 * 2:(j0 + NP // 2) * 2])

        def ma_half(h, half):
            """Half of the z Ma-chain: 4 j-steps of Ma accumulation."""
            s = st[h]
            sh = s["sh"]
            j0 = half * (NP // 2)
            for j in range(j0, j0 + NP // 2):
                for d in range(NT):
                    nc.tensor.matmul(
                        zbig[:, sh, d:d + 1], mt8[:, j, :, bass.ts(d, 128)],
                        s["a8p"][:, j, :, :],
                        start=(j == 0 and d == 0),
                        stop=(j == NP - 1 and d == NT - 1),
                        perf_mode=DR, skip_group_check=True)

        def z_chain(h):
            """z = b^T(M a8), rz = 1/(V z), bp = b*rz."""
            s = st[h]
            sh, b_st = s["sh"], s["b_st"]
            zcol = stpool.tile([128, 1], F32, tag="zc", name=f"zc{h}")
            zscr = stpool.tile([128, NT], F32, tag="zs", name=f"zs{h}")
            nc.vector.scalar_tensor_tensor(
                zscr[:], zbig[:, sh, 0:NT], 1.0, b_st[:],
                op0=MUL, op1=MUL, accum_out=zcol[:])
            nc.tensor.matmul(zbig[0:1, sh, 32:33], ones4[:], zcol[:],
                             start=False, stop=True, skip_group_check=True)
            nc.vector.reciprocal(z1[0:1, h:h + 1], zbig[0:1, sh, 32:33])
            nc.tensor.matmul(zbig[:, sh, 33:34], ones_row[:], z1[0:1, h:h + 1],
                             start=False, stop=True, skip_group_check=True)
            bp = stpool.tile([128, NT], F32, tag="bp", name=f"bp{h}")
            nc.vector.tensor_scalar(bp[:], b_st[:], zbig[:, sh, 33:34], None,
                                    op0=MUL)
            s["bp"] = bp

        pgs = {}

        def g_epi(h, d, split=None):
            """Lagged epilogue for d-tile: out_acc[d] (+)= bp[d] * pg."""
            bp = st[h]["bp"]
            pg = (pgs.pop((h, d)) if split in (None, 1)
                  else pgs[(h, d)])
            last = h == V - 1
            prev = biasb[:] if h == 0 else out_acc[:, d, :]
            cols = slice(None)
            if split is not None:
                cols = slice(0, 448) if split == 0 else slice(448, O)
            if last:
                outf = tmpool.tile([128, O], BF16, tag="outf",
                                   name=f"of{d}_{split}")
            if d % 2 == 1 and not last:
                # ACT-mul + Pool-add path
                tmpd = tmpool.tile([128, O], BF16, tag="tmpd", name=f"tm{h}_{d}")
                nc.scalar.mul(tmpd[:], pg[:], bp[:, d:d + 1])
                nc.gpsimd.tensor_tensor(out_acc[:, d, :], tmpd[:],
                                        prev[:], op=ADD)
            else:
                nc.vector.scalar_tensor_tensor(
                    outf[:, cols] if last else out_acc[:, d, cols],
                    pg[:, cols], bp[:, d:d + 1],
                    prev[:, cols],
                    op0=MUL, op1=ADD)
            if last:
                nc.sync.dma_start(out_d[d, :, cols], outf[:, cols])

        # quantization tile schedule per G d-step: 2 tiles/step early so the
        # next view's quant finishes well before the window boundary
        QSCHED = {0: (0, 1), 1: (2,), 2: (3, 4), 3: (5,),
                  4: (6, 7), 5: (8,), 6: (9, 10), 7: (11,), 8: (12,),
                  9: (13,), 10: (14,), 11: (15,)}

        def g_tile(h, d, hn):
            """B-phase d-tile: 8 main + 8 residual DR matmuls into one bank,
            then the next view's A-phase tiles, then the lagged epilogue.
            The very last tile runs as two column-group chains so its
            epilogue + out-DMA overlap the second chain."""
            s = st[h]
            pg = ppg.tile([128, O], F32, tag="pg", name=f"pg{h}_{d}")
            pgs[(h, d)] = pg
            blk = bass.ts(d, 128)
            for j in range(NP):
                nc.tensor.matmul(pg[:], mt8[:, j, :, blk],
                                 s["haug8"][:, j, :, :],
                                 start=(j == 0), stop=False, perf_mode=DR)
                nc.tensor.matmul(pg[:], mt8[:, j, :, blk],
                                 s["r8"][:, j, :, :],
                                 start=False, stop=(j == NP - 1),
                                 perf_mode=DR)
            if hn is not None:
                for t in QSCHED.get(d, ()):
                    a_tile(hn, t)
                if d == 11:
                    a8p_copy(hn, 0)
                    ma_half(hn, 0)
                if d == 12:
                    a8p_copy(hn, 1)
            if d > 0:
                g_epi(h, d - 1)
            if d == NT - 1:
                if h == V - 1:
                    g_epi(h, d, split=0)
                    g_epi(h, d, split=1)
                else:
                    g_epi(h, d)
                if hn is not None:
                    a_close(hn)

        # ================= trace =================
        a_open(0)
        for t in range(NT):
            a_tile(0, t)
        a8p_copy(0, 0)
        ma_half(0, 0)
        a8p_copy(0, 1)
        a_close(0)
        for h in range(V):
            if h + 1 < V:
                a_open(h + 1)
            for d in range(NT):
                g_tile(h, d, h + 1 if h + 1 < V else None)
                if d == 0:
                    z_chain(h)

    nc.compile()
    _dedup_ldweights(nc)
    return nc


def _dedup_ldweights(nc):
    """Drop InstLdweights that reload the weights AP already resident (the
    main+residual matmul pairs share one mt8 block)."""
    pe = mybir.EngineType.PE
    removed = 0
    for bb in nc.m.functions[0].blocks:
        insts = list(bb.instructions)
        out = []
        last_key = None
        for i in insts:
            ty = type(i).__name__
            if ty == "InstLdweights":
                ap = i.ins[0]
                key = (str(ap.memref), ap.offset, str(ap.ap))
                si = i.sync_info
                clean = si is None or (not si.on_wait and not si.on_update)
                if key == last_key and clean:
                    removed += 1
                    continue
                last_key = key
            elif getattr(i, "engine", None) == pe:
                if ty == "InstMatmult":
                    try:
                        ap = i.ins[1]
                        mk = (str(ap.memref), ap.offset, str(ap.ap))
                    except Exception:
                        mk = None
                    if mk != last_key:
                        last_key = None
                else:
                    last_key = None
            out.append(i)
        if removed:
            bb.instructions = out
    return removed


_SIGNS = None


def _signs():
    global _SIGNS
    if _SIGNS is None:
        s = np.ones((4, F), dtype=np.float32)
        for r in range(4):
            if r & 1:
                s[r, [0, 2]] = -1.0
            if r & 2:
                s[r, [1, 3]] = -1.0
        _SIGNS = s
    return _SIGNS


def _q8pair(x):
    """fp8 hi/lo split of an array (returns e4m3 hi, e4m3 residual)."""
    hi = x.astype(E4M3)
    lo = (x - hi.astype(np.float32)).astype(E4M3)
    return hi, lo


def _host_prep(x, edge_index, W, att, bias):
    """Pure relayout/quantization preprocessing."""
    signs = _signs()
    x = np.ascontiguousarray(x, dtype=np.float32)
    W = np.asarray(W, dtype=np.float32)
    att = np.asarray(att, dtype=np.float32).reshape(2 * O)
    bias = np.asarray(bias, dtype=np.float32)
    ei = np.asarray(edge_index)

    M = np.zeros((N, N), dtype=np.float32)
    np.add.at(M, (ei[1], ei[0]), 1.0)
    M[np.arange(N), np.arange(N)] += 1.0
    # mt8[p, j, i, d] = M[d, (2j+i)*128 + p]
    MT = np.ascontiguousarray(M.T).reshape(NP, 2, 128, N)
    mt8 = np.ascontiguousarray(MT.transpose(2, 0, 1, 3).astype(E4M3))

    att_u, att_v = att[:O], att[O:]
    W1, W2 = W[:F], W[F:]
    attb = np.ascontiguousarray(
        np.broadcast_to(att, (128, 2 * O))).astype(ml_dtypes.bfloat16)
    biasb = np.ascontiguousarray(
        np.broadcast_to(bias, (128, O))).astype(ml_dtypes.bfloat16)

    # bf16 x, transposed to [B, V, F, N]
    xT = np.ascontiguousarray(x.transpose(0, 1, 3, 2)).astype(
        ml_dtypes.bfloat16).astype(np.float32)

    in_maps = []
    for core in range(8):
        b, g = divmod(core, V)
        x8a = np.empty((V, 2, 128, N), dtype=E4M3)
        x8r = np.empty((V, 2, 128, N), dtype=E4M3)
        w8a = np.empty((V, 2, 128, O), dtype=E4M3)
        w8r = np.empty((V, 2, 128, O), dtype=E4M3)
        wuvc = np.empty((V, 2, 128, 2), dtype=ml_dtypes.bfloat16)
        for h in range(V):
            w1s = signs[h ^ g][:, None] * W1
            w2s = signs[h][:, None] * W2
            w1s16 = w1s.astype(ml_dtypes.bfloat16).astype(np.float32)
            w2s16 = w2s.astype(ml_dtypes.bfloat16).astype(np.float32)
            x8a[h, 0], x8r[h, 0] = _q8pair(xT[b, h])
            x8a[h, 1], x8r[h, 1] = _q8pair(xT[b, g ^ h])
            w8a[h, 0], w8r[h, 0] = _q8pair(w1s16)
            w8a[h, 1], w8r[h, 1] = _q8pair(w2s16)
            wuvc[h, 0, :, 0] = 0.6 * (w1s @ att_u)
            wuvc[h, 0, :, 1] = 0.6 * (w1s @ att_v)
            wuvc[h, 1, :, 0] = 0.6 * (w2s @ att_u)
            wuvc[h, 1, :, 1] = 0.6 * (w2s @ att_v)
        in_maps.append({
            "x8a": x8a, "x8r": x8r, "w8a": w8a, "w8r": w8r, "wuv": wuvc,
            "mt8": mt8, "attb": attb, "biasb": biasb,
        })
    return in_maps


_NC = None


def kernel(x, edge_index, W, att, bias):
    global _NC
    if _NC is None:
        _NC = _build_program()
    in_maps = _host_prep(x, edge_index, W, att, bias)

    from concourse.bass_utils import run_bass_kernel_spmd

    res = run_bass_kernel_spmd(_NC, in_maps, list(range(8)))
    out = np.empty((B, V, N, O), dtype=np.float32)
    for core in range(8):
        b, g = divmod(core, V)
        out[b, g] = np.asarray(res.results[core]["out"], dtype=np.float32).reshape(N, O)
    return out
